# revision 1
# baseline (speedup 1.0000x reference)
"""Trainium2 Bass kernel for nn_CrossEntropyLoss_22419729285187.

Computes  -sum_{matched, non-BG true rows} dot(y_true[i,1:], y_pred[rank_i]) / count
sharded over 8 NeuronCores.

Strategy (per sharding hint): the host performs the cheap key join
(encode + searchsorted + cumsum) and compacts to the m_eff matched
(true,pred) row pairs — the r-th matched true row pairs positionally
with y_pred_features[r], so only the true side needs a gather and BG
rows are zeroed in place. The matched pairs are row-sharded across the
8 cores; each core streams its [rows, 32]+[rows, 32]+mask shard
(~19.5MB, large contiguous DMA tiles on both HWDGE rings + SWDGE) and
runs one fused multiply-reduce (scalar_tensor_tensor) per tile on the
DVE, accumulating per-tile partial sums into columns reduced once at
the end. Per-core [128, 2] partials (num, count) are summed on the
host for the final -num/k.

Measured on trn2 x8: ~66-76us HW exec, rel err ~1.5e-6.
"""

import os
import sys

for _p in ("/opt/trn_rl_repo", "/root/.axon_site/_ro/trn_rl_repo"):
    if os.path.isdir(_p) and _p not in sys.path:
        sys.path.append(_p)

import numpy as np

N_CORES = 8

# Device-side tiling: rows are laid out [tile t][partition p][group g];
# each of the 128 partitions owns G consecutive rows per tile.
PARTS = 128
G = 64  # rows per partition per tile (main segment)

_compiled = {}
_last_results = None


def _encode(idx):
    idx = idx.astype(np.int64)
    return ((idx[:, 0] * 1024 + idx[:, 1]) * 1024 + idx[:, 2]) * 1024 + idx[:, 3]


def _build_program(segments, c_pred):
    """Build + schedule the SPMD Tile program for one core shard.

    segments: list of (n_tiles, G) — the shard's rows are laid out
    [tile][partition][group] per segment, concatenated. Using a small
    trailing segment keeps zero-padding minimal while the main segment
    uses large (1MB) DMA tiles.
    """
    from concourse import bacc
    import concourse.mybir as mybir
    from concourse.tile import TileContext

    f32 = mybir.dt.float32
    r_pad = sum(nt * PARTS * g for nt, g in segments)
    total_tiles = sum(nt for nt, _ in segments)

    nc = bacc.Bacc("TRN2", target_bir_lowering=False, debug=False,
                   num_devices=N_CORES)
    yt_d = nc.dram_tensor("yt", [r_pad, c_pred], f32, kind="ExternalInput")
    yp_d = nc.dram_tensor("ypal", [r_pad, c_pred], f32, kind="ExternalInput")
    ax_d = nc.dram_tensor("aux", [r_pad, 1], f32, kind="ExternalInput")
    out_d = nc.dram_tensor("partials", [PARTS, 2], f32, kind="ExternalOutput")

    ax_w = r_pad // PARTS
    with TileContext(nc) as tc:
        with tc.tile_pool(name="acc", bufs=1) as accp:
            red_all = accp.tile([PARTS, total_tiles], f32)
            num_acc = accp.tile([PARTS, 1], f32)
            k_acc = accp.tile([PARTS, 1], f32)
            # k: row order is irrelevant for a global count — one flat
            # [128, r_pad/128] load + one fused square-reduce.
            ax_t = accp.tile([PARTS, ax_w], f32)
            kscr = accp.tile([PARTS, ax_w], f32)
            ax_flat = ax_d.ap().rearrange("(p w) c -> p (w c)", p=PARTS)
            nc.gpsimd.dma_start(out=ax_t[:], in_=ax_flat)
            nc.vector.scalar_tensor_tensor(
                out=kscr[:], in0=ax_t[:], scalar=1.0, in1=ax_t[:],
                op0=mybir.AluOpType.mult, op1=mybir.AluOpType.mult,
                accum_out=k_acc[:])
            with tc.tile_pool(name="io", bufs=5) as pool, \
                 tc.tile_pool(name="scrp", bufs=2) as scrp:
                row0 = 0
                ti = 0
                for nt, g in segments:
                    seg_rows = nt * PARTS * g
                    yt_v = yt_d.ap()[row0:row0 + seg_rows, :].rearrange(
                        "(t p g) c -> t p (g c)", p=PARTS, g=g)
                    yp_v = yp_d.ap()[row0:row0 + seg_rows, :].rearrange(
                        "(t p g) c -> t p (g c)", p=PARTS, g=g)
                    row0 += seg_rows
                    for t in range(nt):
                        yt_t = pool.tile([PARTS, g * c_pred], f32, tag="yt")
                        yp_t = pool.tile([PARTS, g * c_pred], f32, tag="yp")
                        nc.sync.dma_start(out=yt_t[:], in_=yt_v[t])
                        nc.scalar.dma_start(out=yp_t[:], in_=yp_v[t])
                        scr = scrp.tile([PARTS, g * c_pred], f32, tag="scr")
                        # red_all[:, ti] = sum_{g,c} yt * ypal
                        nc.vector.scalar_tensor_tensor(
                            out=scr[:], in0=yt_t[:], scalar=1.0, in1=yp_t[:],
                            op0=mybir.AluOpType.mult, op1=mybir.AluOpType.mult,
                            accum_out=red_all[:, ti:ti + 1])
                        ti += 1
            nc.vector.tensor_reduce(out=num_acc[:], in_=red_all[:],
                                    axis=mybir.AxisListType.X,
                                    op=mybir.AluOpType.add)
            nc.sync.dma_start(out=out_d[:, 0:1], in_=num_acc[:])
            nc.sync.dma_start(out=out_d[:, 1:2], in_=k_acc[:])
    nc.compile()
    return nc


def kernel(y_true_features, y_true_indices, y_pred_features, y_pred_indices):
    global _last_results
    from concourse.bass_utils import run_bass_kernel_spmd

    yt = np.ascontiguousarray(np.asarray(y_true_features, dtype=np.float32))
    yp = np.ascontiguousarray(np.asarray(y_pred_features, dtype=np.float32))
    n, c1 = yt.shape
    m, c = yp.shape

    # ---- host-side key join (cheap integer work) ----
    kt = _encode(np.asarray(y_true_indices))
    kp = _encode(np.asarray(y_pred_indices))
    kps = np.sort(kp)
    pos = np.clip(np.searchsorted(kps, kt), 0, m - 1)
    matched = kps[pos] == kt
    # Only matched true rows contribute to num and k. The r-th matched
    # true row (row order) pairs with y_pred_features[r] positionally
    # (rank = cumsum(matched)-1 is sequential over matched rows), so the
    # pred side needs no gather at all — just the first m_eff rows.
    midx = np.flatnonzero(matched)
    m_eff = midx.size
    yt_cmp = yt[midx, 1:]                      # [m_eff, c] gather
    notbg = yt[midx, 0] != 1.0
    yt_cmp[~notbg] = 0.0                       # BG pairs contribute 0
    aux = notbg.astype(np.float32)

    # ---- shard the m_eff matched pairs across cores ----
    rows = -(-m_eff // N_CORES)
    big = PARTS * G
    nt1 = rows // big
    rem = rows - nt1 * big
    g2 = -(-rem // PARTS)
    segments = ((nt1, G), (1, g2)) if g2 > 0 else ((nt1, G),)
    r_pad = sum(nt * PARTS * g for nt, g in segments)

    key = (segments, c)
    if key not in _compiled:
        _compiled[key] = _build_program(segments, c)
    nc = _compiled[key]

    in_maps = []
    for i in range(N_CORES):
        lo, hi = i * rows, min((i + 1) * rows, m_eff)
        nr = max(hi - lo, 0)
        yt_c = np.zeros((r_pad, c), dtype=np.float32)
        yt_c[:nr] = yt_cmp[lo:hi]
        yp_c = np.zeros((r_pad, c), dtype=np.float32)
        yp_c[:nr] = yp[lo:hi]
        ax_c = np.zeros((r_pad, 1), dtype=np.float32)
        ax_c[:nr, 0] = aux[lo:hi]
        in_maps.append({"yt": yt_c, "ypal": yp_c, "aux": ax_c})

    res = run_bass_kernel_spmd(nc, in_maps, list(range(N_CORES)))
    _last_results = res

    num = 0.0
    k = 0.0
    for i in range(N_CORES):
        p = res.results[i]["partials"]
        num += float(p[:, 0].sum(dtype=np.float64))
        k += float(p[:, 1].sum(dtype=np.float64))
    return np.float32(-num / k)



# revision 2
# speedup vs baseline: 1.6547x; 1.6547x over previous
"""Trainium2 Bass kernel for nn_CrossEntropyLoss_22419729285187.

Computes  -sum_{matched, non-BG true rows} dot(y_true[i,1:], y_pred[rank_i]) / count
sharded over 8 NeuronCores.

Strategy (per sharding hint): the host performs the cheap key join
(encode + searchsorted) and compacts to the matched AND non-background
(true,pred) row pairs — the r-th matched true row pairs positionally
with y_pred_features[r], so dropping BG rows keeps the pairing and the
count k is known on host. The pairs are cast to fp16 (the final scalar
tolerates ~3e-4 rel err; accumulation stays fp32 on device) and
row-sharded across the 8 cores; each core streams its [rows, 32]+[rows,
32] fp16 shard (~8.3MB, large contiguous DMA tiles) and runs one fused
multiply-reduce (scalar_tensor_tensor, fp32 accumulate) per tile on the
DVE, accumulating per-tile partials into columns reduced once at the
end. Per-core [128, 1] num partials are summed on the host for the
final -num/k.
"""

import os
import sys

for _p in ("/opt/trn_rl_repo", "/root/.axon_site/_ro/trn_rl_repo"):
    if os.path.isdir(_p) and _p not in sys.path:
        sys.path.append(_p)

import numpy as np

N_CORES = 8

# Device-side tiling: rows are laid out [tile t][partition p][group g];
# each of the 128 partitions owns G consecutive rows per tile.
PARTS = 128
NT = 8  # tiles per tensor per core

_compiled = {}
_last_results = None


def _encode(idx):
    idx = idx.astype(np.int64)
    return ((idx[:, 0] * 1024 + idx[:, 1]) * 1024 + idx[:, 2]) * 1024 + idx[:, 3]


def _build_program(nt, g, c_pred):
    """Build + schedule the SPMD Tile program for one core shard.

    The shard's rows are laid out [tile][partition][group]: nt tiles,
    128 partitions, g rows per partition per tile, c_pred features per
    row (contiguous g*c_pred fp16 run per partition per tile).
    """
    from concourse import bacc
    import concourse.mybir as mybir
    from concourse.tile import TileContext

    f16 = mybir.dt.float16
    f32 = mybir.dt.float32
    r_pad = nt * PARTS * g

    nc = bacc.Bacc("TRN2", target_bir_lowering=False, debug=False,
                   num_devices=N_CORES)
    yt_d = nc.dram_tensor("yt", [r_pad, c_pred], f16, kind="ExternalInput")
    yp_d = nc.dram_tensor("ypal", [r_pad, c_pred], f16, kind="ExternalInput")
    out_d = nc.dram_tensor("partials", [PARTS, 1], f32, kind="ExternalOutput")

    with TileContext(nc) as tc:
        with tc.tile_pool(name="acc", bufs=1) as accp:
            red_all = accp.tile([PARTS, nt], f32)
            num_acc = accp.tile([PARTS, 1], f32)
            yt_v = yt_d.ap().rearrange("(t p g) c -> t p (g c)", p=PARTS, g=g)
            yp_v = yp_d.ap().rearrange("(t p g) c -> t p (g c)", p=PARTS, g=g)
            with tc.tile_pool(name="io", bufs=5) as pool, \
                 tc.tile_pool(name="scrp", bufs=2) as scrp:
                for t in range(nt):
                    yt_t = pool.tile([PARTS, g * c_pred], f16, tag="yt")
                    yp_t = pool.tile([PARTS, g * c_pred], f16, tag="yp")
                    nc.sync.dma_start(out=yt_t[:], in_=yt_v[t])
                    nc.scalar.dma_start(out=yp_t[:], in_=yp_v[t])
                    scr = scrp.tile([PARTS, g * c_pred], f16, tag="scr")
                    # red_all[:, t] = sum_{g,c} yt * ypal  (fp32 accum)
                    nc.vector.scalar_tensor_tensor(
                        out=scr[:], in0=yt_t[:], scalar=1.0, in1=yp_t[:],
                        op0=mybir.AluOpType.mult, op1=mybir.AluOpType.mult,
                        accum_out=red_all[:, t:t + 1])
            nc.vector.tensor_reduce(out=num_acc[:], in_=red_all[:],
                                    axis=mybir.AxisListType.X,
                                    op=mybir.AluOpType.add)
            nc.sync.dma_start(out=out_d[:, 0:1], in_=num_acc[:])
    nc.compile()
    return nc


def kernel(y_true_features, y_true_indices, y_pred_features, y_pred_indices):
    global _last_results
    from concourse.bass_utils import run_bass_kernel_spmd

    yt = np.ascontiguousarray(np.asarray(y_true_features, dtype=np.float32))
    yp = np.ascontiguousarray(np.asarray(y_pred_features, dtype=np.float32))
    n, c1 = yt.shape
    m, c = yp.shape

    # ---- host-side key join (cheap integer work) ----
    kt = _encode(np.asarray(y_true_indices))
    kp = _encode(np.asarray(y_pred_indices))
    kps = np.sort(kp)
    pos = np.clip(np.searchsorted(kps, kt), 0, m - 1)
    matched = kps[pos] == kt
    # Only matched non-BG true rows contribute. The r-th matched true
    # row (row order) pairs with y_pred_features[r] positionally (rank
    # = cumsum(matched)-1 is sequential over matched rows). Dropping BG
    # rows from both sides keeps the pairing; k is then known here.
    midx = np.flatnonzero(matched)
    nb = yt[midx, 0] != 1.0                      # non-BG mask over matched rows
    k = int(nb.sum())
    A = yt[midx[nb], 1:].astype(np.float16)      # [k, c]
    B = yp[:midx.size][nb].astype(np.float16)    # [k, c]

    # ---- shard the k contributing pairs across cores ----
    rows = -(-k // N_CORES)
    g = -(-rows // (NT * PARTS))
    r_pad = NT * PARTS * g

    key = (NT, g, c)
    if key not in _compiled:
        _compiled[key] = _build_program(NT, g, c)
    nc = _compiled[key]

    in_maps = []
    for i in range(N_CORES):
        lo, hi = i * rows, min((i + 1) * rows, k)
        nr = max(hi - lo, 0)
        yt_c = np.zeros((r_pad, c), dtype=np.float16)
        yt_c[:nr] = A[lo:hi]
        yp_c = np.zeros((r_pad, c), dtype=np.float16)
        yp_c[:nr] = B[lo:hi]
        in_maps.append({"yt": yt_c, "ypal": yp_c})

    res = run_bass_kernel_spmd(nc, in_maps, list(range(N_CORES)))
    _last_results = res

    num = 0.0
    for i in range(N_CORES):
        p = res.results[i]["partials"]
        num += float(p[:, 0].sum(dtype=np.float64))
    return np.float32(-num / k)


# revision 3
# speedup vs baseline: 1.9526x; 1.1800x over previous
"""Trainium2 Bass kernel for nn_CrossEntropyLoss_22419729285187.

Computes  -sum_{matched, non-BG true rows} dot(y_true[i,1:], y_pred[rank_i]) / count
sharded over 8 NeuronCores.

Strategy (per sharding hint): the host performs the cheap key join
(encode + searchsorted) and compacts to the matched AND non-background
(true,pred) row pairs — the r-th matched true row pairs positionally
with y_pred_features[r], so dropping BG rows keeps the pairing and the
count k is known on host. The pairs are cast to fp16 (the final scalar
tolerates ~3e-4 rel err; accumulation stays fp32 on device) and
row-sharded across the 8 cores; each core streams its [rows, 32]+[rows,
32] fp16 shard (~8.3MB, large contiguous DMA tiles) and runs one fused
multiply-reduce (scalar_tensor_tensor, fp32 accumulate) per tile on the
DVE. Tile sizes descend so the last tile's compute tail is short; each
tile's [128, 1] partial is written out immediately after its STT (the
SBUF->DRAM queue is warmed by an early dummy write) so the end-of-
program drain isn't stuck behind a cold write queue. Host sums the
[128, n_tiles] partials from all cores for the final -num/k.
"""

import os
import sys

for _p in ("/opt/trn_rl_repo", "/root/.axon_site/_ro/trn_rl_repo"):
    if os.path.isdir(_p) and _p not in sys.path:
        sys.path.append(_p)

import numpy as np

N_CORES = 8
PARTS = 128

# Rows per partition per tile, descending: big tiles stream with large
# contiguous DMA chunks; the small final tiles shorten the last
# DMA->STT->out dependency chain at the end of the program.
TILE_GS = (126, 126, 126, 77, 32, 16)  # sum = 503 = ceil(64283/128)

_compiled = {}
_last_results = None


def _encode(idx):
    idx = idx.astype(np.int64)
    return ((idx[:, 0] * 1024 + idx[:, 1]) * 1024 + idx[:, 2]) * 1024 + idx[:, 3]


def _build_program(gs, c_pred):
    """Build + schedule the SPMD Tile program for one core shard.

    The shard's rows are laid out tile-by-tile, [partition][group]
    within each tile: tile t holds PARTS*gs[t] rows, each partition
    owning gs[t] consecutive rows (a contiguous gs[t]*c_pred fp16 run).
    """
    from concourse import bacc
    import concourse.mybir as mybir
    from concourse.tile import TileContext

    f16 = mybir.dt.float16
    f32 = mybir.dt.float32
    nt = len(gs)
    r_pad = PARTS * sum(gs)

    nc = bacc.Bacc("TRN2", target_bir_lowering=False, debug=False,
                   num_devices=N_CORES)
    yt_d = nc.dram_tensor("yt", [r_pad, c_pred], f16, kind="ExternalInput")
    yp_d = nc.dram_tensor("ypal", [r_pad, c_pred], f16, kind="ExternalInput")
    out_d = nc.dram_tensor("partials", [PARTS, nt], f32, kind="ExternalOutput")
    warm_d = nc.dram_tensor("warm", [1, 1], f32, kind="ExternalOutput")

    with TileContext(nc) as tc:
        with tc.tile_pool(name="acc", bufs=1) as accp:
            red_all = accp.tile([PARTS, nt], f32)
            warm = accp.tile([1, 1], f32)
            # Touch the SBUF->DRAM write path before it matters: the
            # final partial writes must not hit a cold DMA queue.
            nc.vector.memset(warm[:], 0.0)
            nc.sync.dma_start(out=warm_d[:, :], in_=warm[:])
            with tc.tile_pool(name="io", bufs=5) as pool, \
                 tc.tile_pool(name="scrp", bufs=2) as scrp:
                row0 = 0
                for t, g in enumerate(gs):
                    seg = PARTS * g
                    yt_v = yt_d.ap()[row0:row0 + seg, :].rearrange(
                        "(p g) c -> p (g c)", p=PARTS, g=g)
                    yp_v = yp_d.ap()[row0:row0 + seg, :].rearrange(
                        "(p g) c -> p (g c)", p=PARTS, g=g)
                    row0 += seg
                    yt_t = pool.tile([PARTS, g * c_pred], f16, tag="yt")
                    yp_t = pool.tile([PARTS, g * c_pred], f16, tag="yp")
                    nc.sync.dma_start(out=yt_t[:], in_=yt_v)
                    nc.scalar.dma_start(out=yp_t[:], in_=yp_v)
                    scr = scrp.tile([PARTS, g * c_pred], f16, tag="scr")
                    # red_all[:, t] = sum_{g,c} yt * ypal  (fp32 accum)
                    nc.vector.scalar_tensor_tensor(
                        out=scr[:], in0=yt_t[:], scalar=1.0, in1=yp_t[:],
                        op0=mybir.AluOpType.mult, op1=mybir.AluOpType.mult,
                        accum_out=red_all[:, t:t + 1])
                    # Stream each partial out as soon as it exists: the
                    # last write is tiny and the queue stays warm.
                    nc.sync.dma_start(out=out_d[:, t:t + 1],
                                      in_=red_all[:, t:t + 1])
    nc.compile()
    return nc


def kernel(y_true_features, y_true_indices, y_pred_features, y_pred_indices):
    global _last_results
    from concourse.bass_utils import run_bass_kernel_spmd

    yt = np.ascontiguousarray(np.asarray(y_true_features, dtype=np.float32))
    yp = np.ascontiguousarray(np.asarray(y_pred_features, dtype=np.float32))
    n, c1 = yt.shape
    m, c = yp.shape

    # ---- host-side key join (cheap integer work) ----
    kt = _encode(np.asarray(y_true_indices))
    kp = _encode(np.asarray(y_pred_indices))
    kps = np.sort(kp)
    pos = np.clip(np.searchsorted(kps, kt), 0, m - 1)
    matched = kps[pos] == kt
    # Only matched non-BG true rows contribute. The r-th matched true
    # row (row order) pairs with y_pred_features[r] positionally (rank
    # = cumsum(matched)-1 is sequential over matched rows). Dropping BG
    # rows from both sides keeps the pairing; k is then known here.
    midx = np.flatnonzero(matched)
    nb = yt[midx, 0] != 1.0                      # non-BG mask over matched rows
    k = int(nb.sum())
    A = yt[midx[nb], 1:].astype(np.float16)      # [k, c]
    B = yp[:midx.size][nb].astype(np.float16)    # [k, c]

    # ---- shard the k contributing pairs across cores ----
    rows = -(-k // N_CORES)
    gsum = -(-rows // PARTS)
    gs = list(TILE_GS)
    if sum(gs) < gsum:  # grow the big tiles if the shard is larger
        gs[0] += gsum - sum(gs)
    else:               # shrink from the front to fit
        over = sum(gs) - gsum
        for i in range(len(gs)):
            take = min(over, gs[i] - 1)
            gs[i] -= take
            over -= take
        gs = [g for g in gs if g > 0]
    gs = tuple(gs)
    r_pad = PARTS * sum(gs)

    key = (gs, c)
    if key not in _compiled:
        _compiled[key] = _build_program(gs, c)
    nc = _compiled[key]

    in_maps = []
    for i in range(N_CORES):
        lo, hi = i * rows, min((i + 1) * rows, k)
        nr = max(hi - lo, 0)
        yt_c = np.zeros((r_pad, c), dtype=np.float16)
        yt_c[:nr] = A[lo:hi]
        yp_c = np.zeros((r_pad, c), dtype=np.float16)
        yp_c[:nr] = B[lo:hi]
        in_maps.append({"yt": yt_c, "ypal": yp_c})

    res = run_bass_kernel_spmd(nc, in_maps, list(range(N_CORES)))
    _last_results = res

    num = 0.0
    for i in range(N_CORES):
        p = res.results[i]["partials"]
        num += float(p.sum(dtype=np.float64))
    return np.float32(-num / k)
